# revision 6
# baseline (speedup 1.0000x reference)
"""Sparse-attention kernel for Trainium2 (8 NeuronCores, data-parallel over batch).

Reference computation (L=2048, B=128, H=300):
    proj[l,b,k]   = sum_h qv[l,b,h] * W[k,h] + bias[k]
    energies[b,l] = sum_k proj[l,b,k] * hidden[k,b]
    attn          = softmax(energies, axis=-1)[None]

Algebraic reduction:
    energies[b,l] = sum_h qv[l,b,h] * Wh[h,b] + c[b],  Wh = W^T @ hidden.
c[b] is constant over l so it cancels in softmax; bias is ignored.

Implementation notes:
  * Wh (300x16 per core) is computed on the host in fp32, broadcast to a
    [128, 4800] row-replicated table, and uploaded as fp16 (one small DMA).
  * qv is uploaded as fp16: halves the HBM stream (the roofline) and the
    fp16 rounding perturbs each energy by ~1e-2 absolute, well inside the
    softmax tolerance.  All accumulation stays fp32.
  * Per l-chunk of 128 (16 chunks): DVE runs fused multiply+row-sum
    (scalar_tensor_tensor with accum_out) for most batches; Pool (gpsimd)
    computes the elementwise product for NPOOL_OF[ch] batches and ACT
    accumulates those via activation+accum_out.  Early chunks are
    DVE-heavy because Pool/ACT start late (whb-pool + first product).
    PE transposes each chunk's energies into a PSUM [16, 2048] row
    layout; ACT exponentiates per 4 chunks with per-quad row sums; the
    tail is reciprocal + split scale (ACT 768 cols, DVE 1280) + two
    output DMAs.
  * Raw Bass (manual semaphores): the walrus codegen used by the
    axon/bass2jax path rejects multi-inline-sync-wait instructions and
    some custom-ISA ops (tensor_tensor_reduce, Pool TensorScalarPtr);
    everything used here was verified against that backend.
"""

import sys

if "/opt/trn_rl_repo" not in sys.path:
    sys.path.insert(0, "/opt/trn_rl_repo")

import numpy as np

L, B, H = 2048, 128, 300
NCORES = 8
BL = B // NCORES        # 16 batches per core
P = 128                 # SBUF partitions / l-chunk size
NCH = L // P            # 16 chunks
NSLOT = 6               # qt buffer slots
ESHIFT = -80.0          # static softmax shift (energies lie in [-98, 98])
CW = BL * H             # 4800 columns per chunk
# batches per chunk handled by Pool+ACT (rest fused on DVE); chunk 0 and
# chunk 1 are DVE-heavy because Pool/ACT pipelines start late
NPOOL_OF = [0, 6, 6, 6, 6, 6, 6, 6, 6, 6, 6, 6, 6, 6, 6, 6]
NP15 = NPOOL_OF[NCH - 1]
SC_ACT = 768            # final-scale columns on ACT (DVE takes the rest)

_cache = {}


def _build_nc():
    import concourse.bass as bass
    from concourse import mybir

    f32 = mybir.dt.float32
    f16 = mybir.dt.float16
    Alu = mybir.AluOpType
    Act = mybir.ActivationFunctionType

    nc = bass.Bass("TRN2", target_bir_lowering=False, debug=False)

    qv = nc.dram_tensor("qv", [L, BL, H], f16, kind="ExternalInput").ap()
    whb_d = nc.dram_tensor("whb", [P, CW], f16, kind="ExternalInput").ap()
    ident_d = nc.dram_tensor("ident", [P, P], f32, kind="ExternalInput").ap()
    out = nc.dram_tensor("out", [BL, L], f32, kind="ExternalOutput").ap()

    # --- persistent SBUF tensors
    whb = nc.alloc_sbuf_tensor("whb_t", [P, CW], f16).ap()
    ident = nc.alloc_sbuf_tensor("ident_t", [P, P], f32).ap()
    qth = [nc.alloc_sbuf_tensor(f"qt{s}", [P, CW], f16) for s in range(NSLOT)]
    qt = [h.ap() for h in qth]
    e_all = nc.alloc_sbuf_tensor("e_all", [P, NCH * BL], f32).ap()
    xT = nc.alloc_sbuf_tensor("xT", [BL, L], f32).ap()
    nmx = nc.alloc_sbuf_tensor("nmx", [BL, 1], f32).ap()
    ssum_p = nc.alloc_sbuf_tensor("ssum_p", [BL, NCH // 4], f32).ap()
    ssum = nc.alloc_sbuf_tensor("ssum", [BL, 1], f32).ap()
    rs = nc.alloc_sbuf_tensor("rs", [BL, 1], f32).ap()

    eTp = nc.psum_tensor("eTp", [BL, L], f32).__enter__().ap()

    # --- semaphores
    SDW = [nc.alloc_semaphore(f"SDW{i}") for i in range(3)]  # whb STT pieces
    SDP = nc.alloc_semaphore("SDP")      # whb pool-part
    SDI = nc.alloc_semaphore("SDI")      # identity
    SQZ = [nc.alloc_semaphore(f"SQZ{i}") for i in range(3)]  # ch0 STT pieces
    SQP = nc.alloc_semaphore("SQP")      # ch0 deferred-batch piece
    SQ = [nc.alloc_semaphore(f"SQS{s}") for s in range(NSLOT)]
    SQF = [nc.alloc_semaphore(f"SQF{i}") for i in range(3)]  # ch15 pieces
    SV = nc.alloc_semaphore("SV")        # DVE chunks 1..15 done (in order)
    SVZ = nc.alloc_semaphore("SVZ")      # DVE chunk-0 deferred batches done
    SPo = nc.alloc_semaphore("SPo")      # Pool chunk product done
    SA = nc.alloc_semaphore("SA")        # ACT chunk accums done
    SMM = nc.alloc_semaphore("SMM")      # PE transposes
    SXP = nc.alloc_semaphore("SXP")      # exp quads done
    SRS = nc.alloc_semaphore("SRS")      # reciprocal ready
    SX1 = nc.alloc_semaphore("SX1")      # scale piece (ACT)
    SX2 = nc.alloc_semaphore("SX2")      # scale piece (DVE)
    SOUT = nc.alloc_semaphore("SOUT")
    all_sems = [*SDW, SDP, SDI, *SQZ, SQP, *SQ, *SQF, SV, SVZ, SPo, SA,
                SMM, SXP, SRS, SX1, SX2, SOUT]

    sem_final = {s.name: 0 for s in all_sems}

    def inc(inst, sem, n=1):
        sem_final[sem.name] += n
        return inst.then_inc(sem, n)

    # chunk -> slot, and per-slot DMA ordinal for wait thresholds
    slot_of = {ch: ch % NSLOT for ch in range(NCH)}
    sq_thresh = {}
    _cnt = {s: 0 for s in range(NSLOT)}
    for ch in range(1, NCH - 1):
        s = slot_of[ch]
        _cnt[s] += 16
        sq_thresh[ch] = _cnt[s]
    # number of pooled chunks with index <= ch (SA/SPo wait thresholds)
    pool_idx = {}
    _pc = 0
    for ch in range(NCH):
        if NPOOL_OF[ch] > 0:
            _pc += 1
        pool_idx[ch] = _pc

    with nc.Block() as block:

        @block.sync
        def _(sync: bass.BassEngine):
            # interleave whb pieces with chunk-0 pieces: DVE starts on
            # batches 6..8 as soon as the first two small DMAs land
            inc(sync.dma_start(out=whb[:, 6 * H : 9 * H],
                               in_=whb_d[:, 6 * H : 9 * H]), SDW[0], 16)
            inc(sync.dma_start(out=qt[0][:, 6 * H : 9 * H],
                               in_=qv[0:P, 6:9, :]), SQZ[0], 16)
            inc(sync.dma_start(out=whb[:, 9 * H : 12 * H],
                               in_=whb_d[:, 9 * H : 12 * H]), SDW[1], 16)
            inc(sync.dma_start(out=qt[0][:, 9 * H : 12 * H],
                               in_=qv[0:P, 9:12, :]), SQZ[1], 16)
            inc(sync.dma_start(out=whb[:, 12 * H :],
                               in_=whb_d[:, 12 * H :]), SDW[2], 16)
            inc(sync.dma_start(out=qt[0][:, 12 * H :],
                               in_=qv[0:P, 12:, :]), SQZ[2], 16)
            inc(sync.dma_start(out=qt[1], in_=qv[P : 2 * P, :, :]), SQ[1], 16)
            # pool columns of whb + chunk 0's deferred batches
            inc(sync.dma_start(out=whb[:, : 6 * H],
                               in_=whb_d[:, : 6 * H]), SDP, 16)
            inc(sync.dma_start(out=qt[0][:, : 6 * H],
                               in_=qv[0:P, 0:6, :]), SQP, 16)
            inc(sync.dma_start(out=qt[2], in_=qv[2 * P : 3 * P, :, :]), SQ[2], 16)
            inc(sync.dma_start(out=ident, in_=ident_d), SDI, 16)
            for ch in (3, 4, 5):
                inc(sync.dma_start(
                    out=qt[ch], in_=qv[ch * P : (ch + 1) * P, :, :]
                ), SQ[ch], 16)
            for ch in range(NSLOT, NCH - 1):
                # slot reuse: chunk ch-NSLOT's consumers must be done
                prev = ch - NSLOT
                if prev == 0:
                    sync.wait_ge(SVZ, 1)
                else:
                    sync.wait_ge(SV, prev)
                    if NPOOL_OF[prev] > 0:
                        sync.wait_ge(SA, pool_idx[prev])
                inc(sync.dma_start(
                    out=qt[slot_of[ch]], in_=qv[ch * P : (ch + 1) * P, :, :]
                ), SQ[slot_of[ch]], 16)
            # chunk 15 in three pieces (slot 3; prior user is chunk 9)
            sync.wait_ge(SV, 9)
            sync.wait_ge(SA, pool_idx[9])
            s15 = slot_of[NCH - 1]
            inc(sync.dma_start(out=qt[s15][:, : NP15 * H],
                               in_=qv[15 * P :, 0:NP15, :]), SQF[0], 16)
            inc(sync.dma_start(out=qt[s15][:, NP15 * H : 14 * H],
                               in_=qv[15 * P :, NP15:14, :]), SQF[1], 16)
            inc(sync.dma_start(out=qt[s15][:, 14 * H :],
                               in_=qv[15 * P :, 14:, :]), SQF[2], 16)
            sync.wait_ge(SX1, 1)
            inc(sync.dma_start(out=out[:, :SC_ACT], in_=xT[:, :SC_ACT]),
                SOUT, 16)
            sync.wait_ge(SX2, 1)
            inc(sync.dma_start(out=out[:, SC_ACT:], in_=xT[:, SC_ACT:]),
                SOUT, 16)

        @block.vector
        def _(dve: bass.BassEngine):
            dve.memset(nmx, ESHIFT)

            def stt(ch, b):
                sl = qt[slot_of[ch]][:, b * H : (b + 1) * H]
                dve.scalar_tensor_tensor(
                    out=sl, in0=sl, scalar=1.0,
                    in1=whb[:, b * H : (b + 1) * H],
                    op0=Alu.mult, op1=Alu.mult,
                    accum_out=e_all[:, ch * BL + b : ch * BL + b + 1],
                )

            # chunk 0 front pieces (batches 6..15)
            dve.wait_ge(SDW[0], 16)
            dve.wait_ge(SQZ[0], 16)
            for b in (6, 7, 8):
                stt(0, b)
            dve.wait_ge(SDW[1], 16)
            dve.wait_ge(SQZ[1], 16)
            for b in (9, 10, 11):
                stt(0, b)
            dve.wait_ge(SDW[2], 16)
            dve.wait_ge(SQZ[2], 16)
            for b in (12, 13, 14, 15):
                stt(0, b)
            # chunk 1 next (its DMA lands before chunk 0's deferred piece)
            dve.wait_ge(SQ[slot_of[1]], sq_thresh[1])
            for b in range(NPOOL_OF[1], BL):
                stt(1, b)
            inc(dve.drain(), SV)  # SV=1: chunk 1 done
            # chunk 0 deferred batches 0..5
            dve.wait_ge(SDP, 16)
            dve.wait_ge(SQP, 16)
            for b in range(6):
                stt(0, b)
            inc(dve.drain(), SVZ)
            # chunks 2..14
            for ch in range(2, NCH - 1):
                dve.wait_ge(SQ[slot_of[ch]], sq_thresh[ch])
                for b in range(NPOOL_OF[ch], BL):
                    stt(ch, b)
                inc(dve.drain(), SV)  # SV=ch
            # chunk 15 piecewise
            dve.wait_ge(SQF[1], 16)
            for b in range(NP15, 14):
                stt(15, b)
            dve.wait_ge(SQF[2], 16)
            for b in (14, 15):
                stt(15, b)
            inc(dve.drain(), SV)  # SV=15
            # softmax tail: total row sum, reciprocal, scale the wide piece
            dve.wait_ge(SXP, NCH // 4)
            dve.tensor_reduce(out=ssum, in_=ssum_p,
                              axis=mybir.AxisListType.X, op=Alu.add)
            dve.drain()
            dve.reciprocal(rs, ssum)
            inc(dve.drain(), SRS)
            dve.tensor_scalar_mul(xT[:, SC_ACT:], xT[:, SC_ACT:], rs)
            inc(dve.drain(), SX2)

        @block.gpsimd
        def _(gp: bass.BassEngine):
            gp.wait_ge(SDP, 16)
            for ch in range(NCH):
                np_ = NPOOL_OF[ch]
                if np_ == 0:
                    continue
                if ch == NCH - 1:
                    gp.wait_ge(SQF[0], 16)
                elif ch > 0:
                    gp.wait_ge(SQ[slot_of[ch]], sq_thresh[ch])
                s = slot_of[ch]
                pc = np_ * H
                gp.tensor_mul(qt[s][:, :pc], qt[s][:, :pc], whb[:, :pc])
                inc(gp.drain(), SPo)  # SPo = pool_idx[ch]
            gp.wait_ge(SOUT, 32)

        @block.scalar
        def _(act: bass.BassEngine):
            for ch in range(NCH):
                np_ = NPOOL_OF[ch]
                if np_ > 0:
                    act.wait_ge(SPo, pool_idx[ch])
                    for b in range(np_):
                        sl = qt[slot_of[ch]][:, b * H : (b + 1) * H]
                        act.activation(
                            sl, sl, Act.Copy,
                            accum_out=e_all[:, ch * BL + b : ch * BL + b + 1],
                        )
                    inc(act.drain(), SA)  # SA = pool_idx[ch]
                if ch % 4 == 3:
                    k = ch // 4
                    act.wait_ge(SMM, ch + 1)
                    inc(act.activation(
                        xT[:, k * 4 * P : (k + 1) * 4 * P],
                        eTp[:, k * 4 * P : (k + 1) * 4 * P],
                        Act.Exp, bias=nmx, scale=1.0,
                        accum_out=ssum_p[:, k : k + 1],
                    ), SXP)
            act.wait_ge(SRS, 1)
            act.mul(xT[:, :SC_ACT], xT[:, :SC_ACT], rs)
            inc(act.drain(), SX1)

        @block.tensor
        def _(pe: bass.BassEngine):
            pe.wait_ge(SDI, 16)
            for ch in range(NCH):
                if ch == 0:
                    pe.wait_ge(SVZ, 1)  # batches 0..5 of chunk 0
                else:
                    pe.wait_ge(SV, ch)
                if NPOOL_OF[ch] > 0:
                    pe.wait_ge(SA, pool_idx[ch])
                inc(pe.transpose(
                    eTp[:, ch * P : (ch + 1) * P],
                    e_all[:, ch * BL : (ch + 1) * BL],
                    ident,
                ), SMM)

        nc.all_engine_barrier()
        for s in all_sems:
            if sem_final[s.name]:
                nc.gpsimd.sem_inc(s, -sem_final[s.name])

    return nc


def _get_nc():
    if "nc" not in _cache:
        _cache["nc"] = _build_nc()
    return _cache["nc"]


def make_in_maps(hidden, question_vector, W):
    hidden = np.asarray(hidden, dtype=np.float32)
    qv16 = np.asarray(question_vector, dtype=np.float16)
    W = np.asarray(W, dtype=np.float32)
    wh = W.T @ hidden  # [H, B] fp32
    ident = np.eye(P, dtype=np.float32)
    in_maps = []
    for i in range(NCORES):
        sl = slice(i * BL, (i + 1) * BL)
        whb = np.broadcast_to(
            np.ascontiguousarray(wh[:, sl].T).reshape(1, CW), (P, CW)
        ).astype(np.float16)
        in_maps.append(
            {
                "qv": np.ascontiguousarray(qv16[:, sl, :]),
                "whb": whb,
                "ident": ident,
            }
        )
    return in_maps


def kernel(hidden, question_vector, W, b=None, **kwargs):
    from concourse.bass_utils import run_bass_kernel_spmd

    nc = _get_nc()
    in_maps = make_in_maps(hidden, question_vector, W)
    res = run_bass_kernel_spmd(nc, in_maps, list(range(NCORES)))
    _cache["last_results"] = res
    outs = [np.asarray(res.results[i]["out"]) for i in range(NCORES)]
    attn = np.concatenate(outs, axis=0)[None]
    return np.ascontiguousarray(attn.astype(np.float32))


# revision 8
# speedup vs baseline: 1.0388x; 1.0388x over previous
"""Sparse-attention kernel for Trainium2 (8 NeuronCores, data-parallel over batch).

Reference computation (L=2048, B=128, H=300):
    proj[l,b,k]   = sum_h qv[l,b,h] * W[k,h] + bias[k]
    energies[b,l] = sum_k proj[l,b,k] * hidden[k,b]
    attn          = softmax(energies, axis=-1)[None]

Algebraic reduction:
    energies[b,l] = sum_h qv[l,b,h] * Wh[h,b] + c[b],  Wh = W^T @ hidden.
c[b] is constant over l so it cancels in softmax; bias is ignored.

Implementation notes:
  * Wh (300x16 per core) is computed on the host in fp32, broadcast to a
    [128, 4800] row-replicated table, and uploaded as fp16 (one small DMA).
  * qv is uploaded as fp16: halves the HBM stream (the roofline) and the
    fp16 rounding perturbs each energy by ~1e-2 absolute, well inside the
    softmax tolerance.  All accumulation stays fp32.
  * Per l-chunk of 128 (16 chunks): DVE runs fused multiply+row-sum
    (scalar_tensor_tensor with accum_out) for most batches; Pool (gpsimd)
    computes the elementwise product for NPOOL_OF[ch] batches and ACT
    accumulates those via activation+accum_out.  Early chunks are
    DVE-heavy because Pool/ACT start late (whb-pool + first product).
    PE transposes each chunk's energies into a PSUM [16, 2048] row
    layout; ACT exponentiates per 4 chunks with per-quad row sums; the
    tail is reciprocal + split scale (ACT 768 cols, DVE 1280) + two
    output DMAs.
  * Raw Bass (manual semaphores): the walrus codegen used by the
    axon/bass2jax path rejects multi-inline-sync-wait instructions and
    some custom-ISA ops (tensor_tensor_reduce, Pool TensorScalarPtr);
    everything used here was verified against that backend.
"""

import sys

if "/opt/trn_rl_repo" not in sys.path:
    sys.path.insert(0, "/opt/trn_rl_repo")

import numpy as np

L, B, H = 2048, 128, 300
NCORES = 8
BL = B // NCORES        # 16 batches per core
P = 128                 # SBUF partitions / l-chunk size
NCH = L // P            # 16 chunks
NSLOT = 6               # qt buffer slots
ESHIFT = -80.0          # static softmax shift (energies lie in [-98, 98])
CW = BL * H             # 4800 columns per chunk
# batches per chunk handled by Pool+ACT (rest fused on DVE); chunk 0 and
# chunk 1 are DVE-heavy because Pool/ACT pipelines start late
NPOOL_OF = [0, 4, 5, 5, 6, 6, 6, 6, 6, 6, 6, 6, 6, 6, 5, 5]
NP15 = NPOOL_OF[NCH - 1]
SC_ACT = 768            # final-scale columns on ACT (DVE takes the rest)

_cache = {}


def _build_nc():
    import concourse.bass as bass
    from concourse import mybir

    f32 = mybir.dt.float32
    f16 = mybir.dt.float16
    Alu = mybir.AluOpType
    Act = mybir.ActivationFunctionType

    nc = bass.Bass("TRN2", target_bir_lowering=False, debug=False)

    qv = nc.dram_tensor("qv", [L, BL, H], f16, kind="ExternalInput").ap()
    whb_d = nc.dram_tensor("whb", [P, CW], f16, kind="ExternalInput").ap()
    ident_d = nc.dram_tensor("ident", [P, P], f32, kind="ExternalInput").ap()
    out = nc.dram_tensor("out", [BL, L], f32, kind="ExternalOutput").ap()

    # --- persistent SBUF tensors
    whb = nc.alloc_sbuf_tensor("whb_t", [P, CW], f16).ap()
    ident = nc.alloc_sbuf_tensor("ident_t", [P, P], f32).ap()
    qth = [nc.alloc_sbuf_tensor(f"qt{s}", [P, CW], f16) for s in range(NSLOT)]
    qt = [h.ap() for h in qth]
    e_all = nc.alloc_sbuf_tensor("e_all", [P, NCH * BL], f32).ap()
    xT = nc.alloc_sbuf_tensor("xT", [BL, L], f32).ap()
    nmx = nc.alloc_sbuf_tensor("nmx", [BL, 1], f32).ap()
    ssum_p = nc.alloc_sbuf_tensor("ssum_p", [BL, NCH // 4], f32).ap()
    ssum = nc.alloc_sbuf_tensor("ssum", [BL, 1], f32).ap()
    rs = nc.alloc_sbuf_tensor("rs", [BL, 1], f32).ap()

    eTp = nc.psum_tensor("eTp", [BL, L], f32).__enter__().ap()

    # --- semaphores
    SDW = [nc.alloc_semaphore(f"SDW{i}") for i in range(3)]  # whb STT pieces
    SDP = nc.alloc_semaphore("SDP")      # whb pool-part
    SDI = nc.alloc_semaphore("SDI")      # identity
    SQZ = [nc.alloc_semaphore(f"SQZ{i}") for i in range(3)]  # ch0 STT pieces
    SQP = nc.alloc_semaphore("SQP")      # ch0 deferred-batch piece
    SQ = [nc.alloc_semaphore(f"SQS{s}") for s in range(NSLOT)]
    SQF = [nc.alloc_semaphore(f"SQF{i}") for i in range(3)]  # ch15 pieces
    SV = nc.alloc_semaphore("SV")        # DVE chunks 1..15 done (in order)
    SVZ = nc.alloc_semaphore("SVZ")      # DVE chunk-0 deferred batches done
    SPo = nc.alloc_semaphore("SPo")      # Pool chunk product done
    SA = nc.alloc_semaphore("SA")        # ACT chunk accums done
    SMM = nc.alloc_semaphore("SMM")      # PE transposes
    SXP = nc.alloc_semaphore("SXP")      # exp quads done
    SRS = nc.alloc_semaphore("SRS")      # reciprocal ready
    SX1 = nc.alloc_semaphore("SX1")      # scale piece (ACT)
    SX2 = nc.alloc_semaphore("SX2")      # scale piece (DVE)
    SOUT = nc.alloc_semaphore("SOUT")
    all_sems = [*SDW, SDP, SDI, *SQZ, SQP, *SQ, *SQF, SV, SVZ, SPo, SA,
                SMM, SXP, SRS, SX1, SX2, SOUT]

    sem_final = {s.name: 0 for s in all_sems}

    def inc(inst, sem, n=1):
        sem_final[sem.name] += n
        return inst.then_inc(sem, n)

    # chunk -> slot, and per-slot DMA ordinal for wait thresholds
    slot_of = {ch: ch % NSLOT for ch in range(NCH)}
    sq_thresh = {}
    _cnt = {s: 0 for s in range(NSLOT)}
    for ch in range(1, NCH - 1):
        s = slot_of[ch]
        _cnt[s] += 16
        sq_thresh[ch] = _cnt[s]
    # number of pooled chunks with index <= ch (SA/SPo wait thresholds)
    pool_idx = {}
    _pc = 0
    for ch in range(NCH):
        if NPOOL_OF[ch] > 0:
            _pc += 1
        pool_idx[ch] = _pc

    with nc.Block() as block:

        @block.sync
        def _(sync: bass.BassEngine):
            # interleave whb pieces with chunk-0 pieces: DVE starts on
            # batches 6..8 as soon as the first two small DMAs land
            inc(sync.dma_start(out=whb[:, 6 * H : 9 * H],
                               in_=whb_d[:, 6 * H : 9 * H]), SDW[0], 16)
            inc(sync.dma_start(out=qt[0][:, 6 * H : 9 * H],
                               in_=qv[0:P, 6:9, :]), SQZ[0], 16)
            inc(sync.dma_start(out=whb[:, 9 * H : 12 * H],
                               in_=whb_d[:, 9 * H : 12 * H]), SDW[1], 16)
            inc(sync.dma_start(out=qt[0][:, 9 * H : 12 * H],
                               in_=qv[0:P, 9:12, :]), SQZ[1], 16)
            inc(sync.dma_start(out=whb[:, 12 * H :],
                               in_=whb_d[:, 12 * H :]), SDW[2], 16)
            inc(sync.dma_start(out=qt[0][:, 12 * H :],
                               in_=qv[0:P, 12:, :]), SQZ[2], 16)
            inc(sync.dma_start(out=qt[1], in_=qv[P : 2 * P, :, :]), SQ[1], 16)
            # pool columns of whb + chunk 0's deferred batches
            inc(sync.dma_start(out=whb[:, : 6 * H],
                               in_=whb_d[:, : 6 * H]), SDP, 16)
            inc(sync.dma_start(out=qt[0][:, : 6 * H],
                               in_=qv[0:P, 0:6, :]), SQP, 16)
            inc(sync.dma_start(out=qt[2], in_=qv[2 * P : 3 * P, :, :]), SQ[2], 16)
            inc(sync.dma_start(out=ident, in_=ident_d), SDI, 16)
            for ch in (3, 4, 5):
                inc(sync.dma_start(
                    out=qt[ch], in_=qv[ch * P : (ch + 1) * P, :, :]
                ), SQ[ch], 16)
            for ch in range(NSLOT, NCH - 1):
                # slot reuse: chunk ch-NSLOT's consumers must be done
                prev = ch - NSLOT
                if prev == 0:
                    sync.wait_ge(SVZ, 1)
                else:
                    sync.wait_ge(SV, prev)
                    if NPOOL_OF[prev] > 0:
                        sync.wait_ge(SA, pool_idx[prev])
                inc(sync.dma_start(
                    out=qt[slot_of[ch]], in_=qv[ch * P : (ch + 1) * P, :, :]
                ), SQ[slot_of[ch]], 16)
            # chunk 15 in three pieces (slot 3; prior user is chunk 9)
            sync.wait_ge(SV, 9)
            sync.wait_ge(SA, pool_idx[9])
            s15 = slot_of[NCH - 1]
            inc(sync.dma_start(out=qt[s15][:, : NP15 * H],
                               in_=qv[15 * P :, 0:NP15, :]), SQF[0], 16)
            inc(sync.dma_start(out=qt[s15][:, NP15 * H : 14 * H],
                               in_=qv[15 * P :, NP15:14, :]), SQF[1], 16)
            inc(sync.dma_start(out=qt[s15][:, 14 * H :],
                               in_=qv[15 * P :, 14:, :]), SQF[2], 16)
            sync.wait_ge(SX1, 1)
            inc(sync.dma_start(out=out[:, :SC_ACT], in_=xT[:, :SC_ACT]),
                SOUT, 16)
            sync.wait_ge(SX2, 1)
            inc(sync.dma_start(out=out[:, SC_ACT:], in_=xT[:, SC_ACT:]),
                SOUT, 16)

        @block.vector
        def _(dve: bass.BassEngine):
            dve.memset(nmx, ESHIFT)

            def stt(ch, b):
                sl = qt[slot_of[ch]][:, b * H : (b + 1) * H]
                dve.scalar_tensor_tensor(
                    out=sl, in0=sl, scalar=1.0,
                    in1=whb[:, b * H : (b + 1) * H],
                    op0=Alu.mult, op1=Alu.mult,
                    accum_out=e_all[:, ch * BL + b : ch * BL + b + 1],
                )

            # chunk 0 front pieces (batches 6..15)
            dve.wait_ge(SDW[0], 16)
            dve.wait_ge(SQZ[0], 16)
            for b in (6, 7, 8):
                stt(0, b)
            dve.wait_ge(SDW[1], 16)
            dve.wait_ge(SQZ[1], 16)
            for b in (9, 10, 11):
                stt(0, b)
            dve.wait_ge(SDW[2], 16)
            dve.wait_ge(SQZ[2], 16)
            for b in (12, 13, 14, 15):
                stt(0, b)
            # chunk 1 next (its DMA lands before chunk 0's deferred piece).
            # batches >= 6 use whb columns already landed (SDW pieces);
            # batches < 6 need the whb pool-part (SDP), so they run last.
            dve.wait_ge(SQ[slot_of[1]], sq_thresh[1])
            for b in range(6, BL):
                stt(1, b)
            dve.wait_ge(SDP, 16)
            for b in range(NPOOL_OF[1], 6):
                stt(1, b)
            inc(dve.drain(), SV)  # SV=1: chunk 1 done
            # chunk 0 deferred batches 0..5
            dve.wait_ge(SQP, 16)
            for b in range(6):
                stt(0, b)
            inc(dve.drain(), SVZ)
            # chunks 2..14
            for ch in range(2, NCH - 1):
                dve.wait_ge(SQ[slot_of[ch]], sq_thresh[ch])
                for b in range(NPOOL_OF[ch], BL):
                    stt(ch, b)
                inc(dve.drain(), SV)  # SV=ch
            # chunk 15 piecewise
            dve.wait_ge(SQF[1], 16)
            for b in range(NP15, 14):
                stt(15, b)
            dve.wait_ge(SQF[2], 16)
            for b in (14, 15):
                stt(15, b)
            inc(dve.drain(), SV)  # SV=15
            # softmax tail: total row sum, reciprocal, scale the wide piece
            dve.wait_ge(SXP, NCH // 4)
            dve.tensor_reduce(out=ssum, in_=ssum_p,
                              axis=mybir.AxisListType.X, op=Alu.add)
            dve.drain()
            dve.reciprocal(rs, ssum)
            inc(dve.drain(), SRS)
            dve.tensor_scalar_mul(xT[:, SC_ACT:], xT[:, SC_ACT:], rs)
            inc(dve.drain(), SX2)

        @block.gpsimd
        def _(gp: bass.BassEngine):
            gp.wait_ge(SDP, 16)
            for ch in range(NCH):
                np_ = NPOOL_OF[ch]
                if np_ == 0:
                    continue
                if ch == NCH - 1:
                    gp.wait_ge(SQF[0], 16)
                elif ch > 0:
                    gp.wait_ge(SQ[slot_of[ch]], sq_thresh[ch])
                s = slot_of[ch]
                pc = np_ * H
                gp.tensor_mul(qt[s][:, :pc], qt[s][:, :pc], whb[:, :pc])
                inc(gp.drain(), SPo)  # SPo = pool_idx[ch]
            gp.wait_ge(SOUT, 32)

        @block.scalar
        def _(act: bass.BassEngine):
            for ch in range(NCH):
                np_ = NPOOL_OF[ch]
                if np_ > 0:
                    act.wait_ge(SPo, pool_idx[ch])
                    for b in range(np_):
                        sl = qt[slot_of[ch]][:, b * H : (b + 1) * H]
                        act.activation(
                            sl, sl, Act.Copy,
                            accum_out=e_all[:, ch * BL + b : ch * BL + b + 1],
                        )
                    inc(act.drain(), SA)  # SA = pool_idx[ch]
                if ch % 4 == 3:
                    k = ch // 4
                    act.wait_ge(SMM, ch + 1)
                    inc(act.activation(
                        xT[:, k * 4 * P : (k + 1) * 4 * P],
                        eTp[:, k * 4 * P : (k + 1) * 4 * P],
                        Act.Exp, bias=nmx, scale=1.0,
                        accum_out=ssum_p[:, k : k + 1],
                    ), SXP)
            act.wait_ge(SRS, 1)
            act.mul(xT[:, :SC_ACT], xT[:, :SC_ACT], rs)
            inc(act.drain(), SX1)

        @block.tensor
        def _(pe: bass.BassEngine):
            pe.wait_ge(SDI, 16)
            for ch in range(NCH):
                if ch == 0:
                    pe.wait_ge(SVZ, 1)  # batches 0..5 of chunk 0
                else:
                    pe.wait_ge(SV, ch)
                if NPOOL_OF[ch] > 0:
                    pe.wait_ge(SA, pool_idx[ch])
                inc(pe.transpose(
                    eTp[:, ch * P : (ch + 1) * P],
                    e_all[:, ch * BL : (ch + 1) * BL],
                    ident,
                ), SMM)

        nc.all_engine_barrier()
        for s in all_sems:
            if sem_final[s.name]:
                nc.gpsimd.sem_inc(s, -sem_final[s.name])

    return nc


def _get_nc():
    if "nc" not in _cache:
        _cache["nc"] = _build_nc()
    return _cache["nc"]


def make_in_maps(hidden, question_vector, W):
    hidden = np.asarray(hidden, dtype=np.float32)
    qv16 = np.asarray(question_vector, dtype=np.float16)
    W = np.asarray(W, dtype=np.float32)
    wh = W.T @ hidden  # [H, B] fp32
    ident = np.eye(P, dtype=np.float32)
    in_maps = []
    for i in range(NCORES):
        sl = slice(i * BL, (i + 1) * BL)
        whb = np.broadcast_to(
            np.ascontiguousarray(wh[:, sl].T).reshape(1, CW), (P, CW)
        ).astype(np.float16)
        in_maps.append(
            {
                "qv": np.ascontiguousarray(qv16[:, sl, :]),
                "whb": whb,
                "ident": ident,
            }
        )
    return in_maps


def kernel(hidden, question_vector, W, b=None, **kwargs):
    from concourse.bass_utils import run_bass_kernel_spmd

    nc = _get_nc()
    in_maps = make_in_maps(hidden, question_vector, W)
    res = run_bass_kernel_spmd(nc, in_maps, list(range(NCORES)))
    _cache["last_results"] = res
    outs = [np.asarray(res.results[i]["out"]) for i in range(NCORES)]
    attn = np.concatenate(outs, axis=0)[None]
    return np.ascontiguousarray(attn.astype(np.float32))
